# revision 47
# baseline (speedup 1.0000x reference)
"""BuddingLayer Trainium2 kernel (8-core, routed + fp8, contiguous-stream).

Reference computation (N = size_in = 8192, O = size_out = 8192):
    mask  = (x * saturated) != 0                   # ~half the neurons
    h2    = tiny per-neuron MLP(x)                              [N,3]
    h3    = relu(sum_i W3[n,o,i] * h2[n,i] + b3[n,o])           [N,O]
    u[o]  = sum_n mask[n] * h3[n,o]
    out   = weight @ (x * ~saturated) + bias + u

Host-side routing gathers the ~4112 active experts (mask=1) and ~4080
active dense columns; all big streams are fp8e4 (tolerance 2e-2).
Per-core stream ~21 MB -> ~60 us DMA floor at 358 GB/s.

v2 design (from 113.5us baseline profile):
  * The baseline spent ~130us of aggregate issue-engine time on ~200
    small strided DMAs (512B packets) and ~25us of Tensor time on bf16
    ones-reduce matmuls.  Both are restructured away:
  * Expert stream is pre-tiled on host into per-o-block CONTIGUOUS
    super-tiles pe[b] = [128, ns, 2(pair), 2(row), 512] fp8 -> ONE
    1 MB DMA per block (16 total), 8 KB/partition contiguous.
  * All ns=4 slabs go through the PE DoubleRow path (2 matmuls/slab
    into stk_s).  relu is a 2-op chain per psum tile pair: Scalar
    ACTIVATE(relu) writes row r of a [128,2,512] fp8 acc tile for slab
    2r, DVE scalar_tensor_tensor max-add accumulates slab 2r+1.
  * One DoubleRow reduce matmul per block contracts acc against a
    0.125-valued block-column selector -> row b of a single [16,512]
    psum bank accumulated across all 16 blocks; one copy + one store
    at the end.  (0.125 = 1/H2S undoes the stationary's 8x scale.)
  * Dense matvec: wt pre-tiled into 8 contiguous pair-tiles
    [128, 2(kc), 2(row), 1024] fp8, DoubleRow with xg stationary.
  * Small MLP consts for the 4 slab groups + 2 tail experts are merged
    (nt=6) into single per-field DMAs.
"""

import sys

import numpy as np

_TRN = "/opt/trn_rl_repo"
if _TRN not in sys.path:
    sys.path.insert(0, _TRN)

import ml_dtypes

import concourse.bacc as bacc
import concourse.mybir as mybir
from concourse import tile
from concourse.bass_utils import run_bass_kernel_spmd

F32 = mybir.dt.float32
BF16 = mybir.dt.bfloat16
FP8 = mybir.dt.float8e4
AF = mybir.ActivationFunctionType
ALU = mybir.AluOpType
AX = mybir.AxisListType
PM = mybir.MatmulPerfMode

NP_BF16 = ml_dtypes.bfloat16
NP_FP8 = ml_dtypes.float8_e4m3

N_CORES = 8
SIZE_IN = 8192
SIZE_OUT = 8192
OC = SIZE_OUT // 128          # o-chunks for the tail layout
O_BLK = 512                   # o-block (one psum bank of f32)
WT_SCALE = 1024.0             # dense weights are subnormal in fp8; prescale
H2S = 8.0                     # h2 scale in the PE stationary (undone in reduce)


def build_program(
    ns,                        # 128-expert PE slabs per core (must be even)
    n_tail,                    # leftover experts per core (o-transposed path)
    nkc2,                      # 256-row DoubleRow chunks for dense (even)
    size_out=SIZE_OUT,
    n_cores=N_CORES,
    pe_bufs=16,
    acc_bufs=8,
    tail_blk=10,
    enable_asserts=False,
):
    assert ns % 2 == 0 and ns >= 2
    m_own = size_out // n_cores
    NB = size_out // O_BLK
    assert nkc2 == NB          # dense kc chunks padded to one per o-block
    nrow = ns // 2
    nt_all = ns + n_tail       # merged small-MLP groups (slab + tail)
    PW = ns * 2 * 2 * O_BLK    # expert payload bytes per partition per block
    DW = 2 * m_own             # dense payload bytes per partition per block

    nc = bacc.Bacc(
        "TRN2",
        target_bir_lowering=False,
        debug=False,
        enable_asserts=enable_asserts,
        num_devices=n_cores,
    )

    d = {}
    # per-block merged stream tile: expert slabs (PW bytes/partition) then
    # the block's dense weight kc-chunk (DW bytes/partition)
    d["pe"] = nc.dram_tensor("pe", [NB, 128, PW + DW], FP8,
                             kind="ExternalInput")
    # ONE packed const tensor (fp8 bytes, bitcast views):
    #   [0:128] cind | [128:+2*nkc2] xg | tbf bf16 bytes | cpk f32 bytes
    TBF_OFF = 128 + 2 * nkc2
    TBF_W = n_tail * 4 * OC * 2
    CPK_OFF = TBF_OFF + TBF_W
    assert CPK_OFF % 4 == 0
    FPK_W = CPK_OFF + 25 * nt_all * 4
    d["fpk"] = nc.dram_tensor("fpk", [128, FPK_W], FP8, kind="ExternalInput")
    if n_tail:
        d["ut_out"] = nc.dram_tensor("ut_out", [128, OC], F32,
                                     kind="ExternalOutput")
    d["u_out"] = nc.dram_tensor("u_out", [NB, O_BLK], F32, kind="ExternalOutput")
    d["dense_out"] = nc.dram_tensor("dense_out", [1, m_own], F32,
                                    kind="ExternalOutput")

    def small_mlp(cp, x_sb, w1, b1, w2, b2, nt, pfx):
        h0 = cp.tile([128, nt], F32, tag=f"{pfx}h0")
        nc.vector.tensor_scalar_mul(h0[:], x_sb[:], 1.0 / 3.0)
        rs1 = cp.tile([128, nt, 3], F32, tag=f"{pfx}rs1")
        nc.vector.tensor_reduce(rs1[:], w1[:], axis=AX.X, op=ALU.add)
        h1 = cp.tile([128, nt, 3], F32, tag=f"{pfx}h1")
        for t in range(nt):
            nc.vector.scalar_tensor_tensor(
                h1[:, t, :], rs1[:, t, :], h0[:, t : t + 1], b1[:, t, :],
                op0=ALU.mult, op1=ALU.add,
            )
        nc.vector.tensor_scalar_max(h1[:], h1[:], 0.0)
        h2 = cp.tile([128, nt, 3], F32, tag=f"{pfx}h2")
        for t in range(nt):
            nc.vector.scalar_tensor_tensor(
                h2[:, t, :], w2[:, t, :, 0], h1[:, t, 0:1], b2[:, t, :],
                op0=ALU.mult, op1=ALU.add,
            )
            for i in (1, 2):
                nc.vector.scalar_tensor_tensor(
                    h2[:, t, :], w2[:, t, :, i], h1[:, t, i : i + 1], h2[:, t, :],
                    op0=ALU.mult, op1=ALU.add,
                )
        nc.vector.tensor_scalar_max(h2[:], h2[:], 0.0)
        return h2

    with tile.TileContext(nc) as tc:
        with (
            tc.tile_pool(name="const", bufs=1) as cp,
            tc.tile_pool(name="pep", bufs=pe_bufs) as pep,
            tc.tile_pool(name="accp", bufs=acc_bufs) as accp,
            tc.tile_pool(name="rp", bufs=2) as rp,
            tc.tile_pool(name="outp", bufs=2) as outp,
            tc.tile_pool(name="pp", bufs=1, space="PSUM") as pp,
        ):
            # ---- single packed const load, FIRST on the sync queue ---------
            # (on its own queue it gets starved behind the pe stream; at the
            # head of sync's queue it lands ~4us and unblocks h2/stationaries)
            nt = nt_all
            fpk = cp.tile([128, FPK_W], FP8)
            nc.sync.dma_start(fpk[:], d["fpk"][:])
            if n_tail:
                tbf = fpk[:, TBF_OFF:CPK_OFF].bitcast(BF16).rearrange(
                    "p (t f c) -> p t f c", t=n_tail, f=4, c=OC)
                ut = cp.tile([128, OC], F32)
            cind = fpk[:, 0:128]
            xg = fpk[:, 128 : 128 + 2 * nkc2].rearrange(
                "p (r k) -> p r k", r=2, k=nkc2)
            cpk = fpk[:, CPK_OFF:FPK_W].bitcast(F32)
            x_all = cpk[:, 0:nt]
            w1 = cpk[:, nt : 10 * nt].rearrange(
                "p (t a b) -> p t a b", t=nt, a=3, b=3)
            b1 = cpk[:, 10 * nt : 13 * nt].rearrange(
                "p (t a) -> p t a", t=nt, a=3)
            w2 = cpk[:, 13 * nt : 22 * nt].rearrange(
                "p (t a b) -> p t a b", t=nt, a=3, b=3)
            b2 = cpk[:, 22 * nt : 25 * nt].rearrange(
                "p (t a) -> p t a", t=nt, a=3)

            # ---- streaming DMA issues, all up-front ------------------------
            # One uniform 1.3MB merged tile per block (experts + dense kc),
            # ALL on the sync hardware-DGE queue: a single hwdge queue
            # sustains the full ~420 GB/s per-core HBM bandwidth, and sync
            # runs no compute, so the issues execute back-to-back in
            # consumption order.  Scalar carries only the one const load
            # (so its relus are never queued behind big DMAs) and gpsimd
            # only the three output stores (its software DGE is too slow
            # for streams, and store hoisting there is harmless).
            pets = [pep.tile([128, PW + DW], FP8, tag="pe", name=f"pet{b}")
                    for b in range(NB)]
            for b in range(NB):
                nc.sync.dma_start(pets[b][:], d["pe"][b : b + 1])

            # ---- reduce selector stationaries (device-built) --------------
            # sel[:, b, r, j] = 0.125 iff j == b : directs block b's 2-row
            # relu-acc contraction into row b of the u psum bank.
            sel = cp.tile([128, NB, nrow, NB], FP8)
            nc.vector.memset(sel[:], 0.0)
            for b in range(NB):
                nc.vector.memset(sel[:, b, :, b : b + 1], 1.0 / H2S)

            # ---- h2 for all experts (slab groups + tail groups) -----------
            h2 = small_mlp(cp, x_all, w1, b1, w2, b2, nt_all, "v")

            # diagonal stationaries: Sa = (8I)*h2_0 | (8I)*h2_1,
            # Sb = (8I)*h2_2 | 8I   (bias row coefficient)
            stat = []
            for s in range(ns):
                Sa = cp.tile([128, 2, 128], FP8, tag=f"Sa{s}")
                Sb = cp.tile([128, 2, 128], FP8, tag=f"Sb{s}")
                for c in (0, 1):
                    nc.vector.tensor_scalar(
                        Sa[:, c, :], cind[:], h2[:, s, c : c + 1], None,
                        op0=ALU.mult,
                    )
                nc.vector.tensor_scalar(
                    Sb[:, 0, :], cind[:], h2[:, s, 2:3], None, op0=ALU.mult,
                )
                nc.vector.tensor_copy(Sb[:, 1, :], cind[:])
                stat.append((Sa, Sb))

            # ---- persistent psum tiles ------------------------------------
            u_all = pp.tile([NB, O_BLK], F32, tag="uall")
            d_psum = pp.tile([1, m_own], F32, tag="dpsum")

            # ---- main streamed loop ---------------------------------------
            pend = []              # (block, acc) awaiting their reduce matmuls
            for b in range(NB):
                pet = pets[b]
                pev = pet[:, 0:PW].rearrange(
                    "p (s a r c) -> p s a r c", s=ns, a=2, r=2, c=O_BLK)
                wtv = pet[:, PW : PW + DW].rearrange(
                    "p (r m) -> p r m", r=2, m=m_own)
                # ---------- PE path: 2 DoubleRow matmuls per slab -----------
                stks = []
                for s in range(ns):
                    stk = pp.tile([128, O_BLK], F32, tag=f"stk{s}")
                    Sa, Sb = stat[s]
                    nc.tensor.matmul(
                        stk[:], Sa[:], pev[:, s, 0, :, :],
                        start=True, stop=False, perf_mode=PM.DoubleRow,
                    )
                    nc.tensor.matmul(
                        stk[:], Sb[:], pev[:, s, 1, :, :],
                        start=False, stop=True, perf_mode=PM.DoubleRow,
                    )
                    stks.append(stk)

                # ---------- dense matvec: this block's kc chunk -------------
                for mb in range(m_own // 512):
                    lo, hi = mb * 512, (mb + 1) * 512
                    nc.tensor.matmul(
                        d_psum[0:1, lo:hi],
                        xg[:, :, b : b + 1],
                        wtv[:, :, lo:hi],
                        start=(b == 0), stop=(b == NB - 1),
                        perf_mode=PM.DoubleRow,
                    )
                if b == NB - 1:
                    # bias is added on the host
                    dense_sb = outp.tile([1, m_own], F32, tag="dense_sb")
                    nc.vector.tensor_scalar_mul(
                        dense_sb[:], d_psum[:], 1.0 / WT_SCALE)
                    nc.sync.dma_start(d["dense_out"][:], dense_sb[:])

                # ---------- per-slab relus, fully independent ---------------
                # (Scalar takes even slabs, DVE odd slabs; no chaining, so
                # the per-block critical path is one relu past the last MM)
                acc = accp.tile([128, ns, O_BLK], FP8, tag="acc")
                for s in range(ns):
                    if s % 2 == 0:
                        nc.scalar.activation(acc[:, s, :], stks[s][:], AF.Relu)
                    else:
                        nc.vector.tensor_scalar_max(
                            acc[:, s, :], stks[s][:], 0.0)
                pend.append((b, acc))

                # ---------- tail experts, once, early ----------
                if n_tail and b == tail_blk:
                    for e in range(n_tail):
                        tacc = rp.tile([128, OC], BF16, tag=f"tacc{e}")
                        nc.vector.scalar_tensor_tensor(
                            tacc[:], tbf[:, e, 0, :], h2[:, ns + e, 0:1],
                            tbf[:, e, 3, :], op0=ALU.mult, op1=ALU.add,
                        )
                        for i in (1, 2):
                            nc.vector.scalar_tensor_tensor(
                                tacc[:], tbf[:, e, i, :], h2[:, ns + e, i : i + 1],
                                tacc[:], op0=ALU.mult, op1=ALU.add,
                            )
                        if e == 0:
                            nc.scalar.activation(ut[:], tacc[:], AF.Relu)
                        else:
                            rt = rp.tile([128, OC], F32, tag="rt")
                            nc.scalar.activation(rt[:], tacc[:], AF.Relu)
                            nc.vector.tensor_tensor(ut[:], ut[:], rt[:], op=ALU.add)
                    nc.sync.dma_start(d["ut_out"][:], ut[:])

            # ---------- all reduces after the loop + single u store ---------
            # Emitted past every block's matmuls: the scheduler hoists each
            # into Tensor slack once its acc is ready, but a lagging relu
            # can never stall the streaming pipeline.  Two DR matmuls per
            # block contract slab-row pairs (0,1) and (2,3) of acc.
            for pb, pacc in pend:
                for h in range(nrow):
                    nc.tensor.matmul(
                        u_all[:], sel[:, pb, :, :],
                        pacc[:, 2 * h : 2 * h + 2, :],
                        start=(pb == 0 and h == 0),
                        stop=(pb == NB - 1 and h == nrow - 1),
                        perf_mode=PM.DoubleRow,
                    )
            u_sb = outp.tile([NB, O_BLK], F32, tag="u_sb")
            nc.vector.tensor_copy(u_sb[:], u_all[:])
            nc.sync.dma_start(d["u_out"][:], u_sb[:])

    nc.compile()
    return nc, d


def route(inputs):
    """Host-side routing: active experts + active dense columns."""
    x = np.asarray(inputs["x"], dtype=np.float32)
    sat = np.asarray(inputs["saturated"]).astype(bool)
    act = np.nonzero(sat & (x != 0))[0]
    dcols = np.nonzero(~sat)[0]
    per = -(-len(act) // N_CORES)            # ceil
    nslab = per // 128                       # full 128-expert slabs
    if nslab % 2:                            # DR reduce pairs slabs
        nslab -= 1
    n_tail = per - 128 * nslab
    nkc2 = -(-len(dcols) // 256)
    if nkc2 % 2:
        nkc2 += 1                            # dense pair-tiles need even kc
    return act, dcols, per, 0, nslab, n_tail, nkc2


def make_in_maps(inputs, act, dcols, per, nsub, nslab, n_tail, nkc2):
    x = np.asarray(inputs["x"], dtype=np.float32)
    weight = np.asarray(inputs["weight"], dtype=np.float32)
    bias = np.asarray(inputs["bias"], dtype=np.float32)
    W1 = np.asarray(inputs["W1"], dtype=np.float32)
    b1 = np.asarray(inputs["b1"], dtype=np.float32)
    W2 = np.asarray(inputs["W2"], dtype=np.float32)
    b2 = np.asarray(inputs["b2"], dtype=np.float32)
    W3 = np.asarray(inputs["W3"], dtype=np.float32)
    b3 = np.asarray(inputs["b3"], dtype=np.float32)

    ns = nslab
    m_own = SIZE_OUT // N_CORES
    NB = SIZE_OUT // O_BLK
    npair = nkc2 // 2
    n_slab = 128 * ns
    Dp = nkc2 * 256

    W38 = W3.astype(NP_FP8)                  # [N, O, 3]
    b38 = b3.astype(NP_FP8)                  # [N, O]

    xg_full = np.zeros(Dp, dtype=np.float32)
    xg_full[: len(dcols)] = x[dcols]
    # DoubleRow pairs: partition p of chunk kc holds rows kc*256+2p, +1
    xg = np.ascontiguousarray(
        xg_full.reshape(nkc2, 128, 2).transpose(1, 2, 0)
    ).astype(NP_FP8)

    cind = (H2S * np.eye(128, dtype=np.float32)).astype(NP_FP8)

    in_maps = []
    for i in range(N_CORES):
        ids = act[i * per : (i + 1) * per]
        n_live = len(ids)
        if n_live < per:
            ids = np.concatenate([ids, np.zeros(per - n_live, dtype=ids.dtype)])
        gids = ids[:n_slab]
        tids = ids[n_slab:]

        # ---- contiguous per-o-block merged stream tiles ------------------
        G = np.empty((n_slab, SIZE_OUT, 4), dtype=NP_FP8)
        G[:, :, 0:3] = W38[gids]
        G[:, :, 3] = b38[gids]
        live = min(max(n_live, 0), n_slab)
        if live < n_slab:
            G[live:] = 0
        PW = ns * 2 * 2 * O_BLK
        pe = np.empty((NB, 128, PW + 2 * m_own), dtype=NP_FP8)
        pe[:, :, 0:PW] = (
            G.reshape(ns, 128, NB, O_BLK, 4).transpose(2, 1, 0, 4, 3)
            .reshape(NB, 128, PW)
        )

        slm = slice(i * m_own, (i + 1) * m_own)
        wtg = np.zeros((Dp, m_own), dtype=np.float32)
        wtg[: len(dcols)] = weight[slm][:, dcols].T * WT_SCALE
        pe[:, :, PW:] = wtg.astype(NP_FP8).reshape(NB, 128, 2 * m_own)

        # ---- merged small-MLP consts (slab groups + tail groups) ---------
        def grp(a, shp):
            main = a[gids].reshape((ns, 128) + shp).transpose(
                (1, 0) + tuple(range(2, 2 + len(shp))))
            if n_tail:
                tailb = np.broadcast_to(a[tids], (128, n_tail) + shp)
                main = np.concatenate([main, tailb], axis=1)
            return main.reshape(128, -1)

        nt = ns + n_tail
        cpkarr = np.ascontiguousarray(np.concatenate(
            [grp(x, ()), grp(W1, (3, 3)), grp(b1, (3,)),
             grp(W2, (3, 3)), grp(b2, (3,))], axis=1, dtype=np.float32))

        TBF_OFF = 128 + 2 * nkc2
        TBF_W = n_tail * 4 * OC * 2
        CPK_OFF = TBF_OFF + TBF_W
        FPK_W = CPK_OFF + 25 * nt * 4
        raw = np.zeros((128, FPK_W), dtype=np.uint8)
        raw[:, 0:128] = cind.view(np.uint8)
        raw[:, 128:TBF_OFF] = xg.reshape(128, 2 * nkc2).view(np.uint8)
        raw[:, CPK_OFF:FPK_W] = cpkarr.view(np.uint8)

        if n_tail:
            nt_live = max(0, min(n_tail, n_live - n_slab))
            w3tt = np.ascontiguousarray(
                W3[tids]
                .transpose(0, 2, 1)
                .reshape(n_tail, 3, OC, 128)
                .transpose(3, 0, 1, 2)
            ).astype(NP_BF16)
            b3tt = np.ascontiguousarray(
                b3[tids].reshape(n_tail, OC, 128).transpose(2, 0, 1)
            ).astype(NP_BF16)
            if nt_live < n_tail:
                w3tt[:, nt_live:] = 0
                b3tt[:, nt_live:] = 0
            tbf = np.empty((128, n_tail, 4, OC), dtype=NP_BF16)
            tbf[:, :, 0:3, :] = w3tt
            tbf[:, :, 3, :] = b3tt
            raw[:, TBF_OFF:CPK_OFF] = tbf.view(np.uint8).reshape(128, TBF_W)

        m = {"pe": pe, "fpk": raw.view(NP_FP8)}
        in_maps.append(m)
    return in_maps


def combine_outputs(results, names, n_tail, bias=None):
    u = np.zeros(SIZE_OUT, dtype=np.float64)
    dense = []
    for res in results:
        u += res[names["u_out"].name].reshape(-1).astype(np.float64)
        if n_tail:
            ut = res[names["ut_out"].name].astype(np.float64)  # [128, OC]
            u += ut.T.reshape(-1)                              # o = c*128 + p
        dense.append(res[names["dense_out"].name].reshape(-1))
    out = np.concatenate(dense).astype(np.float64) + u
    if bias is not None:
        out = out + np.asarray(bias, dtype=np.float64)
    return out.astype(np.float32)


_CACHE = {}
CONFIG = {}


def _get_program(nsub, nslab, n_tail, nkc2):
    key = (nsub, nslab, n_tail, nkc2, tuple(sorted(CONFIG.items())))
    if key not in _CACHE:
        _CACHE[key] = build_program(nslab, n_tail, nkc2, **CONFIG)
    return _CACHE[key]


def kernel(**inputs):
    act, dcols, per, nsub, nslab, n_tail, nkc2 = route(inputs)
    nc, names = _get_program(nsub, nslab, n_tail, nkc2)
    in_maps = make_in_maps(inputs, act, dcols, per, nsub, nslab, n_tail, nkc2)
    keyed = [{names[k].name: v for k, v in m.items()} for m in in_maps]
    res = run_bass_kernel_spmd(nc, keyed, core_ids=list(range(N_CORES)))
    return combine_outputs(res.results, names, n_tail, inputs["bias"])


# revision 49
# speedup vs baseline: 1.0094x; 1.0094x over previous
"""BuddingLayer Trainium2 kernel (8-core, routed + fp8, contiguous-stream).

Reference computation (N = size_in = 8192, O = size_out = 8192):
    mask  = (x * saturated) != 0                   # ~half the neurons
    h2    = tiny per-neuron MLP(x)                              [N,3]
    h3    = relu(sum_i W3[n,o,i] * h2[n,i] + b3[n,o])           [N,O]
    u[o]  = sum_n mask[n] * h3[n,o]
    out   = weight @ (x * ~saturated) + bias + u

Host-side routing gathers the ~4112 active experts (mask=1) and ~4080
active dense columns; all big streams are fp8e4 (tolerance 2e-2).
Per-core stream ~21 MB -> ~60 us DMA floor at 358 GB/s.

v2 design (from 113.5us baseline profile):
  * The baseline spent ~130us of aggregate issue-engine time on ~200
    small strided DMAs (512B packets) and ~25us of Tensor time on bf16
    ones-reduce matmuls.  Both are restructured away:
  * Expert stream is pre-tiled on host into per-o-block CONTIGUOUS
    super-tiles pe[b] = [128, ns, 2(pair), 2(row), 512] fp8 -> ONE
    1 MB DMA per block (16 total), 8 KB/partition contiguous.
  * All ns=4 slabs go through the PE DoubleRow path (2 matmuls/slab
    into stk_s).  relu is a 2-op chain per psum tile pair: Scalar
    ACTIVATE(relu) writes row r of a [128,2,512] fp8 acc tile for slab
    2r, DVE scalar_tensor_tensor max-add accumulates slab 2r+1.
  * One DoubleRow reduce matmul per block contracts acc against a
    0.125-valued block-column selector -> row b of a single [16,512]
    psum bank accumulated across all 16 blocks; one copy + one store
    at the end.  (0.125 = 1/H2S undoes the stationary's 8x scale.)
  * Dense matvec: wt pre-tiled into 8 contiguous pair-tiles
    [128, 2(kc), 2(row), 1024] fp8, DoubleRow with xg stationary.
  * Small MLP consts for the 4 slab groups + 2 tail experts are merged
    (nt=6) into single per-field DMAs.
"""

import sys

import numpy as np

_TRN = "/opt/trn_rl_repo"
if _TRN not in sys.path:
    sys.path.insert(0, _TRN)

import ml_dtypes

import concourse.bacc as bacc
import concourse.mybir as mybir
from concourse import tile
from concourse.bass_utils import run_bass_kernel_spmd

F32 = mybir.dt.float32
BF16 = mybir.dt.bfloat16
FP8 = mybir.dt.float8e4
AF = mybir.ActivationFunctionType
ALU = mybir.AluOpType
AX = mybir.AxisListType
PM = mybir.MatmulPerfMode

NP_BF16 = ml_dtypes.bfloat16
NP_FP8 = ml_dtypes.float8_e4m3

N_CORES = 8
SIZE_IN = 8192
SIZE_OUT = 8192
OC = SIZE_OUT // 128          # o-chunks for the tail layout
O_BLK = 512                   # o-block (one psum bank of f32)
WT_SCALE = 1024.0             # dense weights are subnormal in fp8; prescale
H2S = 8.0                     # h2 scale in the PE stationary (undone in reduce)


def build_program(
    ns,                        # 128-expert PE slabs per core (must be even)
    n_tail,                    # leftover experts per core (o-transposed path)
    nkc2,                      # 256-row DoubleRow chunks for dense (even)
    size_out=SIZE_OUT,
    n_cores=N_CORES,
    pe_bufs=16,
    acc_bufs=8,
    tail_blk=10,
    enable_asserts=False,
):
    assert ns % 2 == 0 and ns >= 2
    m_own = size_out // n_cores
    NB = size_out // O_BLK
    assert nkc2 == NB          # dense kc chunks padded to one per o-block
    nrow = ns // 2
    nt_all = ns + n_tail       # merged small-MLP groups (slab + tail)
    PW = ns * 2 * 2 * O_BLK    # expert payload bytes per partition per block
    DW = 2 * m_own             # dense payload bytes per partition per block

    nc = bacc.Bacc(
        "TRN2",
        target_bir_lowering=False,
        debug=False,
        enable_asserts=enable_asserts,
        num_devices=n_cores,
    )

    d = {}
    # per-block merged stream tile: expert slabs (PW bytes/partition) then
    # the block's dense weight kc-chunk (DW bytes/partition)
    d["pe"] = nc.dram_tensor("pe", [NB, 128, PW + DW], FP8,
                             kind="ExternalInput")
    # ONE packed const tensor (fp8 bytes, bitcast views):
    #   [0:128] cind | [128:+2*nkc2] xg | tbf bf16 bytes | cpk f32 bytes
    TBF_OFF = 128 + 2 * nkc2
    TBF_W = n_tail * 4 * OC * 2
    CPK_OFF = TBF_OFF + TBF_W
    assert CPK_OFF % 4 == 0
    FPK_W = CPK_OFF + 25 * nt_all * 4
    d["fpk"] = nc.dram_tensor("fpk", [128, FPK_W], FP8, kind="ExternalInput")
    if n_tail:
        d["ut_out"] = nc.dram_tensor("ut_out", [128, OC], F32,
                                     kind="ExternalOutput")
    d["u_out"] = nc.dram_tensor("u_out", [NB, O_BLK], F32, kind="ExternalOutput")
    d["dense_out"] = nc.dram_tensor("dense_out", [1, m_own], F32,
                                    kind="ExternalOutput")

    def small_mlp(cp, x_sb, w1, b1, w2, b2, nt, pfx):
        h0 = cp.tile([128, nt], F32, tag=f"{pfx}h0")
        nc.vector.tensor_scalar_mul(h0[:], x_sb[:], 1.0 / 3.0)
        rs1 = cp.tile([128, nt, 3], F32, tag=f"{pfx}rs1")
        nc.vector.tensor_reduce(rs1[:], w1[:], axis=AX.X, op=ALU.add)
        h1 = cp.tile([128, nt, 3], F32, tag=f"{pfx}h1")
        for t in range(nt):
            nc.vector.scalar_tensor_tensor(
                h1[:, t, :], rs1[:, t, :], h0[:, t : t + 1], b1[:, t, :],
                op0=ALU.mult, op1=ALU.add,
            )
        nc.vector.tensor_scalar_max(h1[:], h1[:], 0.0)
        h2 = cp.tile([128, nt, 3], F32, tag=f"{pfx}h2")
        for t in range(nt):
            nc.vector.scalar_tensor_tensor(
                h2[:, t, :], w2[:, t, :, 0], h1[:, t, 0:1], b2[:, t, :],
                op0=ALU.mult, op1=ALU.add,
            )
            for i in (1, 2):
                nc.vector.scalar_tensor_tensor(
                    h2[:, t, :], w2[:, t, :, i], h1[:, t, i : i + 1], h2[:, t, :],
                    op0=ALU.mult, op1=ALU.add,
                )
        nc.vector.tensor_scalar_max(h2[:], h2[:], 0.0)
        return h2

    with tile.TileContext(nc) as tc:
        with (
            tc.tile_pool(name="const", bufs=1) as cp,
            tc.tile_pool(name="pep", bufs=pe_bufs) as pep,
            tc.tile_pool(name="accp", bufs=acc_bufs) as accp,
            tc.tile_pool(name="rp", bufs=2) as rp,
            tc.tile_pool(name="outp", bufs=2) as outp,
            tc.tile_pool(name="pp", bufs=1, space="PSUM") as pp,
        ):
            # ---- single packed const load, FIRST on the sync queue ---------
            # (on its own queue it gets starved behind the pe stream; at the
            # head of sync's queue it lands ~4us and unblocks h2/stationaries)
            nt = nt_all
            fpk = cp.tile([128, FPK_W], FP8)
            nc.sync.dma_start(fpk[:], d["fpk"][:])
            if n_tail:
                tbf = fpk[:, TBF_OFF:CPK_OFF].bitcast(BF16).rearrange(
                    "p (t f c) -> p t f c", t=n_tail, f=4, c=OC)
                ut = cp.tile([128, OC], F32)
            cind = fpk[:, 0:128]
            xg = fpk[:, 128 : 128 + 2 * nkc2].rearrange(
                "p (r k) -> p r k", r=2, k=nkc2)
            cpk = fpk[:, CPK_OFF:FPK_W].bitcast(F32)
            x_all = cpk[:, 0:nt]
            w1 = cpk[:, nt : 10 * nt].rearrange(
                "p (t a b) -> p t a b", t=nt, a=3, b=3)
            b1 = cpk[:, 10 * nt : 13 * nt].rearrange(
                "p (t a) -> p t a", t=nt, a=3)
            w2 = cpk[:, 13 * nt : 22 * nt].rearrange(
                "p (t a b) -> p t a b", t=nt, a=3, b=3)
            b2 = cpk[:, 22 * nt : 25 * nt].rearrange(
                "p (t a) -> p t a", t=nt, a=3)

            # ---- streaming DMA issues, all up-front ------------------------
            # One uniform 1.3MB merged tile per block (experts + dense kc),
            # ALL on the sync hardware-DGE queue: a single hwdge queue
            # sustains the full ~420 GB/s per-core HBM bandwidth, and sync
            # runs no compute, so the issues execute back-to-back in
            # consumption order.  Scalar carries only the one const load
            # (so its relus are never queued behind big DMAs) and gpsimd
            # only the three output stores (its software DGE is too slow
            # for streams, and store hoisting there is harmless).
            pets = [pep.tile([128, PW + DW], FP8, tag="pe", name=f"pet{b}")
                    for b in range(NB)]
            for b in range(NB):
                nc.sync.dma_start(pets[b][:], d["pe"][b : b + 1])

            # ---- reduce selector stationaries (device-built) --------------
            # sel[:, b, r, j] = 0.125 iff j == b : directs block b's 2-row
            # relu-acc contraction into row b of the u psum bank.
            sel = cp.tile([128, NB, nrow, NB], FP8)
            nc.vector.memset(sel[:], 0.0)
            for b in range(NB):
                nc.vector.memset(sel[:, b, :, b : b + 1], 1.0 / H2S)

            # ---- h2 for all experts (slab groups + tail groups) -----------
            h2 = small_mlp(cp, x_all, w1, b1, w2, b2, nt_all, "v")

            # diagonal stationaries: Sa = (8I)*h2_0 | (8I)*h2_1,
            # Sb = (8I)*h2_2 | 8I   (bias row coefficient)
            stat = []
            for s in range(ns):
                Sa = cp.tile([128, 2, 128], FP8, tag=f"Sa{s}")
                Sb = cp.tile([128, 2, 128], FP8, tag=f"Sb{s}")
                for c in (0, 1):
                    nc.vector.tensor_scalar(
                        Sa[:, c, :], cind[:], h2[:, s, c : c + 1], None,
                        op0=ALU.mult,
                    )
                nc.vector.tensor_scalar(
                    Sb[:, 0, :], cind[:], h2[:, s, 2:3], None, op0=ALU.mult,
                )
                nc.vector.tensor_copy(Sb[:, 1, :], cind[:])
                stat.append((Sa, Sb))

            # ---- persistent psum tiles ------------------------------------
            u_all = pp.tile([NB, O_BLK], F32, tag="uall")
            d_psum = pp.tile([1, m_own], F32, tag="dpsum")

            # ---- main streamed loop ---------------------------------------
            pend = []              # (block, acc) awaiting their reduce matmuls
            for b in range(NB):
                pet = pets[b]
                pev = pet[:, 0:PW].rearrange(
                    "p (s a r c) -> p s a r c", s=ns, a=2, r=2, c=O_BLK)
                wtv = pet[:, PW : PW + DW].rearrange(
                    "p (r m) -> p r m", r=2, m=m_own)
                # ---------- PE path: 2 DoubleRow matmuls per slab -----------
                stks = []
                for s in range(ns):
                    stk = pp.tile([128, O_BLK], F32, tag=f"stk{s}")
                    Sa, Sb = stat[s]
                    nc.tensor.matmul(
                        stk[:], Sa[:], pev[:, s, 0, :, :],
                        start=True, stop=False, perf_mode=PM.DoubleRow,
                    )
                    nc.tensor.matmul(
                        stk[:], Sb[:], pev[:, s, 1, :, :],
                        start=False, stop=True, perf_mode=PM.DoubleRow,
                    )
                    stks.append(stk)

                # ---------- dense matvec: this block's kc chunk -------------
                for mb in range(m_own // 512):
                    lo, hi = mb * 512, (mb + 1) * 512
                    nc.tensor.matmul(
                        d_psum[0:1, lo:hi],
                        xg[:, :, b : b + 1],
                        wtv[:, :, lo:hi],
                        start=(b == 0), stop=(b == NB - 1),
                        perf_mode=PM.DoubleRow,
                    )
                if b == NB - 1:
                    # bias is added on the host
                    dense_sb = outp.tile([1, m_own], F32, tag="dense_sb")
                    nc.vector.tensor_scalar_mul(
                        dense_sb[:], d_psum[:], 1.0 / WT_SCALE)
                    nc.gpsimd.dma_start(d["dense_out"][:], dense_sb[:])

                # ---------- per-slab relus, fully independent ---------------
                # (Scalar takes even slabs, DVE odd slabs; no chaining, so
                # the per-block critical path is one relu past the last MM)
                acc = accp.tile([128, ns, O_BLK], FP8, tag="acc")
                for s in range(ns):
                    if s % 2 == 0:
                        nc.scalar.activation(acc[:, s, :], stks[s][:], AF.Relu)
                    else:
                        nc.vector.tensor_scalar_max(
                            acc[:, s, :], stks[s][:], 0.0)
                pend.append((b, acc))

                # ---------- tail experts, once, early ----------
                if n_tail and b == tail_blk:
                    for e in range(n_tail):
                        tacc = rp.tile([128, OC], BF16, tag=f"tacc{e}")
                        nc.vector.scalar_tensor_tensor(
                            tacc[:], tbf[:, e, 0, :], h2[:, ns + e, 0:1],
                            tbf[:, e, 3, :], op0=ALU.mult, op1=ALU.add,
                        )
                        for i in (1, 2):
                            nc.vector.scalar_tensor_tensor(
                                tacc[:], tbf[:, e, i, :], h2[:, ns + e, i : i + 1],
                                tacc[:], op0=ALU.mult, op1=ALU.add,
                            )
                        if e == 0:
                            nc.vector.tensor_scalar_max(ut[:], tacc[:], 0.0)
                        else:
                            rt = rp.tile([128, OC], F32, tag="rt")
                            nc.vector.tensor_scalar_max(rt[:], tacc[:], 0.0)
                            nc.vector.tensor_tensor(ut[:], ut[:], rt[:], op=ALU.add)
                    nc.gpsimd.dma_start(d["ut_out"][:], ut[:])

            # ---------- all reduces after the loop + single u store ---------
            # Emitted past every block's matmuls: the scheduler hoists each
            # into Tensor slack once its acc is ready, but a lagging relu
            # can never stall the streaming pipeline.  Two DR matmuls per
            # block contract slab-row pairs (0,1) and (2,3) of acc.
            for pb, pacc in pend:
                for h in range(nrow):
                    nc.tensor.matmul(
                        u_all[:], sel[:, pb, :, :],
                        pacc[:, 2 * h : 2 * h + 2, :],
                        start=(pb == 0 and h == 0),
                        stop=(pb == NB - 1 and h == nrow - 1),
                        perf_mode=PM.DoubleRow,
                    )
            u_sb = outp.tile([NB, O_BLK], F32, tag="u_sb")
            nc.vector.tensor_copy(u_sb[:], u_all[:])
            nc.gpsimd.dma_start(d["u_out"][:], u_sb[:])

    nc.compile()
    return nc, d


def route(inputs):
    """Host-side routing: active experts + active dense columns."""
    x = np.asarray(inputs["x"], dtype=np.float32)
    sat = np.asarray(inputs["saturated"]).astype(bool)
    act = np.nonzero(sat & (x != 0))[0]
    dcols = np.nonzero(~sat)[0]
    per = -(-len(act) // N_CORES)            # ceil
    nslab = per // 128                       # full 128-expert slabs
    if nslab % 2:                            # DR reduce pairs slabs
        nslab -= 1
    n_tail = per - 128 * nslab
    nkc2 = -(-len(dcols) // 256)
    if nkc2 % 2:
        nkc2 += 1                            # dense pair-tiles need even kc
    return act, dcols, per, 0, nslab, n_tail, nkc2


def make_in_maps(inputs, act, dcols, per, nsub, nslab, n_tail, nkc2):
    x = np.asarray(inputs["x"], dtype=np.float32)
    weight = np.asarray(inputs["weight"], dtype=np.float32)
    bias = np.asarray(inputs["bias"], dtype=np.float32)
    W1 = np.asarray(inputs["W1"], dtype=np.float32)
    b1 = np.asarray(inputs["b1"], dtype=np.float32)
    W2 = np.asarray(inputs["W2"], dtype=np.float32)
    b2 = np.asarray(inputs["b2"], dtype=np.float32)
    W3 = np.asarray(inputs["W3"], dtype=np.float32)
    b3 = np.asarray(inputs["b3"], dtype=np.float32)

    ns = nslab
    m_own = SIZE_OUT // N_CORES
    NB = SIZE_OUT // O_BLK
    npair = nkc2 // 2
    n_slab = 128 * ns
    Dp = nkc2 * 256

    W38 = W3.astype(NP_FP8)                  # [N, O, 3]
    b38 = b3.astype(NP_FP8)                  # [N, O]

    xg_full = np.zeros(Dp, dtype=np.float32)
    xg_full[: len(dcols)] = x[dcols]
    # DoubleRow pairs: partition p of chunk kc holds rows kc*256+2p, +1
    xg = np.ascontiguousarray(
        xg_full.reshape(nkc2, 128, 2).transpose(1, 2, 0)
    ).astype(NP_FP8)

    cind = (H2S * np.eye(128, dtype=np.float32)).astype(NP_FP8)

    in_maps = []
    for i in range(N_CORES):
        ids = act[i * per : (i + 1) * per]
        n_live = len(ids)
        if n_live < per:
            ids = np.concatenate([ids, np.zeros(per - n_live, dtype=ids.dtype)])
        gids = ids[:n_slab]
        tids = ids[n_slab:]

        # ---- contiguous per-o-block merged stream tiles ------------------
        G = np.empty((n_slab, SIZE_OUT, 4), dtype=NP_FP8)
        G[:, :, 0:3] = W38[gids]
        G[:, :, 3] = b38[gids]
        live = min(max(n_live, 0), n_slab)
        if live < n_slab:
            G[live:] = 0
        PW = ns * 2 * 2 * O_BLK
        pe = np.empty((NB, 128, PW + 2 * m_own), dtype=NP_FP8)
        pe[:, :, 0:PW] = (
            G.reshape(ns, 128, NB, O_BLK, 4).transpose(2, 1, 0, 4, 3)
            .reshape(NB, 128, PW)
        )

        slm = slice(i * m_own, (i + 1) * m_own)
        wtg = np.zeros((Dp, m_own), dtype=np.float32)
        wtg[: len(dcols)] = weight[slm][:, dcols].T * WT_SCALE
        pe[:, :, PW:] = wtg.astype(NP_FP8).reshape(NB, 128, 2 * m_own)

        # ---- merged small-MLP consts (slab groups + tail groups) ---------
        def grp(a, shp):
            main = a[gids].reshape((ns, 128) + shp).transpose(
                (1, 0) + tuple(range(2, 2 + len(shp))))
            if n_tail:
                tailb = np.broadcast_to(a[tids], (128, n_tail) + shp)
                main = np.concatenate([main, tailb], axis=1)
            return main.reshape(128, -1)

        nt = ns + n_tail
        cpkarr = np.ascontiguousarray(np.concatenate(
            [grp(x, ()), grp(W1, (3, 3)), grp(b1, (3,)),
             grp(W2, (3, 3)), grp(b2, (3,))], axis=1, dtype=np.float32))

        TBF_OFF = 128 + 2 * nkc2
        TBF_W = n_tail * 4 * OC * 2
        CPK_OFF = TBF_OFF + TBF_W
        FPK_W = CPK_OFF + 25 * nt * 4
        raw = np.zeros((128, FPK_W), dtype=np.uint8)
        raw[:, 0:128] = cind.view(np.uint8)
        raw[:, 128:TBF_OFF] = xg.reshape(128, 2 * nkc2).view(np.uint8)
        raw[:, CPK_OFF:FPK_W] = cpkarr.view(np.uint8)

        if n_tail:
            nt_live = max(0, min(n_tail, n_live - n_slab))
            w3tt = np.ascontiguousarray(
                W3[tids]
                .transpose(0, 2, 1)
                .reshape(n_tail, 3, OC, 128)
                .transpose(3, 0, 1, 2)
            ).astype(NP_BF16)
            b3tt = np.ascontiguousarray(
                b3[tids].reshape(n_tail, OC, 128).transpose(2, 0, 1)
            ).astype(NP_BF16)
            if nt_live < n_tail:
                w3tt[:, nt_live:] = 0
                b3tt[:, nt_live:] = 0
            tbf = np.empty((128, n_tail, 4, OC), dtype=NP_BF16)
            tbf[:, :, 0:3, :] = w3tt
            tbf[:, :, 3, :] = b3tt
            raw[:, TBF_OFF:CPK_OFF] = tbf.view(np.uint8).reshape(128, TBF_W)

        m = {"pe": pe, "fpk": raw.view(NP_FP8)}
        in_maps.append(m)
    return in_maps


def combine_outputs(results, names, n_tail, bias=None):
    u = np.zeros(SIZE_OUT, dtype=np.float64)
    dense = []
    for res in results:
        u += res[names["u_out"].name].reshape(-1).astype(np.float64)
        if n_tail:
            ut = res[names["ut_out"].name].astype(np.float64)  # [128, OC]
            u += ut.T.reshape(-1)                              # o = c*128 + p
        dense.append(res[names["dense_out"].name].reshape(-1))
    out = np.concatenate(dense).astype(np.float64) + u
    if bias is not None:
        out = out + np.asarray(bias, dtype=np.float64)
    return out.astype(np.float32)


_CACHE = {}
CONFIG = {}


def _get_program(nsub, nslab, n_tail, nkc2):
    key = (nsub, nslab, n_tail, nkc2, tuple(sorted(CONFIG.items())))
    if key not in _CACHE:
        _CACHE[key] = build_program(nslab, n_tail, nkc2, **CONFIG)
    return _CACHE[key]


def kernel(**inputs):
    act, dcols, per, nsub, nslab, n_tail, nkc2 = route(inputs)
    nc, names = _get_program(nsub, nslab, n_tail, nkc2)
    in_maps = make_in_maps(inputs, act, dcols, per, nsub, nslab, n_tail, nkc2)
    keyed = [{names[k].name: v for k, v in m.items()} for m in in_maps]
    res = run_bass_kernel_spmd(nc, keyed, core_ids=list(range(N_CORES)))
    return combine_outputs(res.results, names, n_tail, inputs["bias"])


# revision 50
# speedup vs baseline: 1.1198x; 1.1094x over previous
"""BuddingLayer Trainium2 kernel (8-core, routed + fp8, contiguous-stream).

Reference computation (N = size_in = 8192, O = size_out = 8192):
    mask  = (x * saturated) != 0                   # ~half the neurons
    h2    = tiny per-neuron MLP(x)                              [N,3]
    h3    = relu(sum_i W3[n,o,i] * h2[n,i] + b3[n,o])           [N,O]
    u[o]  = sum_n mask[n] * h3[n,o]
    out   = weight @ (x * ~saturated) + bias + u

Host-side routing gathers the ~4112 active experts (mask=1) and ~4080
active dense columns; all big streams are fp8e4 (tolerance 2e-2).
Per-core stream ~21 MB -> ~60 us DMA floor at 358 GB/s.

v2 design (from 113.5us baseline profile):
  * The baseline spent ~130us of aggregate issue-engine time on ~200
    small strided DMAs (512B packets) and ~25us of Tensor time on bf16
    ones-reduce matmuls.  Both are restructured away:
  * Expert stream is pre-tiled on host into per-o-block CONTIGUOUS
    super-tiles pe[b] = [128, ns, 2(pair), 2(row), 512] fp8 -> ONE
    1 MB DMA per block (16 total), 8 KB/partition contiguous.
  * All ns=4 slabs go through the PE DoubleRow path (2 matmuls/slab
    into stk_s).  relu is a 2-op chain per psum tile pair: Scalar
    ACTIVATE(relu) writes row r of a [128,2,512] fp8 acc tile for slab
    2r, DVE scalar_tensor_tensor max-add accumulates slab 2r+1.
  * One DoubleRow reduce matmul per block contracts acc against a
    0.125-valued block-column selector -> row b of a single [16,512]
    psum bank accumulated across all 16 blocks; one copy + one store
    at the end.  (0.125 = 1/H2S undoes the stationary's 8x scale.)
  * Dense matvec: wt pre-tiled into 8 contiguous pair-tiles
    [128, 2(kc), 2(row), 1024] fp8, DoubleRow with xg stationary.
  * Small MLP consts for the 4 slab groups + 2 tail experts are merged
    (nt=6) into single per-field DMAs.
"""

import sys

import numpy as np

_TRN = "/opt/trn_rl_repo"
if _TRN not in sys.path:
    sys.path.insert(0, _TRN)

import ml_dtypes

import concourse.bacc as bacc
import concourse.mybir as mybir
from concourse import tile
from concourse.bass_utils import run_bass_kernel_spmd

F32 = mybir.dt.float32
BF16 = mybir.dt.bfloat16
FP8 = mybir.dt.float8e4
AF = mybir.ActivationFunctionType
ALU = mybir.AluOpType
AX = mybir.AxisListType
PM = mybir.MatmulPerfMode

NP_BF16 = ml_dtypes.bfloat16
NP_FP8 = ml_dtypes.float8_e4m3

N_CORES = 8
SIZE_IN = 8192
SIZE_OUT = 8192
OC = SIZE_OUT // 128          # o-chunks for the tail layout
O_BLK = 512                   # o-block (one psum bank of f32)
WT_SCALE = 1024.0             # dense weights are subnormal in fp8; prescale
H2S = 8.0                     # h2 scale in the PE stationary (undone in reduce)


def build_program(
    ns,                        # 128-expert PE slabs per core (must be even)
    n_tail,                    # leftover experts per core (o-transposed path)
    nkc2,                      # 256-row DoubleRow chunks for dense (even)
    size_out=SIZE_OUT,
    n_cores=N_CORES,
    pe_bufs=16,
    acc_bufs=8,
    tail_blk=10,
    enable_asserts=False,
):
    assert ns % 2 == 0 and ns >= 2
    m_own = size_out // n_cores
    NB = size_out // O_BLK
    assert nkc2 == NB          # dense kc chunks padded to one per o-block
    nrow = ns // 2
    nt_all = ns + n_tail       # merged small-MLP groups (slab + tail)
    PW = ns * 2 * 2 * O_BLK    # expert payload bytes per partition per block
    DW = 2 * m_own             # dense payload bytes per partition per block

    nc = bacc.Bacc(
        "TRN2",
        target_bir_lowering=False,
        debug=False,
        enable_asserts=enable_asserts,
        num_devices=n_cores,
    )

    d = {}
    # per-block merged stream tile: expert slabs (PW bytes/partition) then
    # the block's dense weight kc-chunk (DW bytes/partition)
    d["pe"] = nc.dram_tensor("pe", [NB, 128, PW + DW], FP8,
                             kind="ExternalInput")
    # ONE packed const tensor (fp8 bytes, bitcast views):
    #   [0:128] cind | [128:+2*nkc2] xg | tbf bf16 bytes | cpk f32 bytes
    TBF_OFF = 128 + 2 * nkc2
    TBF_W = n_tail * 4 * OC * 2
    CPK_OFF = TBF_OFF + TBF_W
    assert CPK_OFF % 4 == 0
    FPK_W = CPK_OFF + 25 * nt_all * 4
    d["fpk"] = nc.dram_tensor("fpk", [128, FPK_W], FP8, kind="ExternalInput")
    if n_tail:
        d["ut_out"] = nc.dram_tensor("ut_out", [128, OC], F32,
                                     kind="ExternalOutput")
    d["u_out"] = nc.dram_tensor("u_out", [NB, O_BLK], F32, kind="ExternalOutput")
    d["dense_out"] = nc.dram_tensor("dense_out", [1, m_own], F32,
                                    kind="ExternalOutput")

    def small_mlp(cp, x_sb, w1, b1, w2, b2, nt, pfx):
        h0 = cp.tile([128, nt], F32, tag=f"{pfx}h0")
        nc.vector.tensor_scalar_mul(h0[:], x_sb[:], 1.0 / 3.0)
        rs1 = cp.tile([128, nt, 3], F32, tag=f"{pfx}rs1")
        nc.vector.tensor_reduce(rs1[:], w1[:], axis=AX.X, op=ALU.add)
        h1 = cp.tile([128, nt, 3], F32, tag=f"{pfx}h1")
        for t in range(nt):
            nc.vector.scalar_tensor_tensor(
                h1[:, t, :], rs1[:, t, :], h0[:, t : t + 1], b1[:, t, :],
                op0=ALU.mult, op1=ALU.add,
            )
        nc.vector.tensor_scalar_max(h1[:], h1[:], 0.0)
        h2 = cp.tile([128, nt, 3], F32, tag=f"{pfx}h2")
        for t in range(nt):
            nc.vector.scalar_tensor_tensor(
                h2[:, t, :], w2[:, t, :, 0], h1[:, t, 0:1], b2[:, t, :],
                op0=ALU.mult, op1=ALU.add,
            )
            for i in (1, 2):
                nc.vector.scalar_tensor_tensor(
                    h2[:, t, :], w2[:, t, :, i], h1[:, t, i : i + 1], h2[:, t, :],
                    op0=ALU.mult, op1=ALU.add,
                )
        nc.vector.tensor_scalar_max(h2[:], h2[:], 0.0)
        return h2

    with tile.TileContext(nc) as tc:
        with (
            tc.tile_pool(name="const", bufs=1) as cp,
            tc.tile_pool(name="pep", bufs=pe_bufs) as pep,
            tc.tile_pool(name="accp", bufs=acc_bufs) as accp,
            tc.tile_pool(name="rp", bufs=2) as rp,
            tc.tile_pool(name="outp", bufs=2) as outp,
            tc.tile_pool(name="pp", bufs=1, space="PSUM") as pp,
        ):
            # ---- single packed const load, FIRST on the sync queue ---------
            # (on its own queue it gets starved behind the pe stream; at the
            # head of sync's queue it lands ~4us and unblocks h2/stationaries)
            nt = nt_all
            fpk = cp.tile([128, FPK_W], FP8)
            nc.sync.dma_start(fpk[:], d["fpk"][:])
            if n_tail:
                tbf = fpk[:, TBF_OFF:CPK_OFF].bitcast(BF16).rearrange(
                    "p (t f c) -> p t f c", t=n_tail, f=4, c=OC)
                ut = cp.tile([128, OC], F32)
            cind = fpk[:, 0:128]
            xg = fpk[:, 128 : 128 + 2 * nkc2].rearrange(
                "p (r k) -> p r k", r=2, k=nkc2)
            cpk = fpk[:, CPK_OFF:FPK_W].bitcast(F32)
            x_all = cpk[:, 0:nt]
            w1 = cpk[:, nt : 10 * nt].rearrange(
                "p (t a b) -> p t a b", t=nt, a=3, b=3)
            b1 = cpk[:, 10 * nt : 13 * nt].rearrange(
                "p (t a) -> p t a", t=nt, a=3)
            w2 = cpk[:, 13 * nt : 22 * nt].rearrange(
                "p (t a b) -> p t a b", t=nt, a=3, b=3)
            b2 = cpk[:, 22 * nt : 25 * nt].rearrange(
                "p (t a) -> p t a", t=nt, a=3)

            # ---- streaming DMA issues, all up-front ------------------------
            # One uniform 1.3MB merged tile per block (experts + dense kc),
            # ALL on the sync hardware-DGE queue: a single hwdge queue
            # sustains the full ~420 GB/s per-core HBM bandwidth, and sync
            # runs no compute, so the issues execute back-to-back in
            # consumption order.  Scalar carries only the one const load
            # (so its relus are never queued behind big DMAs) and gpsimd
            # only the three output stores (its software DGE is too slow
            # for streams, and store hoisting there is harmless).
            pets = [pep.tile([128, PW + DW], FP8, tag="pe", name=f"pet{b}")
                    for b in range(NB)]
            for b in range(NB):
                nc.sync.dma_start(pets[b][:], d["pe"][b : b + 1])

            # ---- reduce selector stationaries (device-built) --------------
            # sel[:, b, r, j] = 0.125 iff j == b : directs block b's 2-row
            # relu-acc contraction into row b of the u psum bank.
            sel = cp.tile([128, NB, nrow, NB], FP8)
            nc.vector.memset(sel[:], 0.0)
            for b in range(NB):
                nc.vector.memset(sel[:, b, :, b : b + 1], 1.0 / H2S)

            # ---- h2 for all experts (slab groups + tail groups) -----------
            h2 = small_mlp(cp, x_all, w1, b1, w2, b2, nt_all, "v")

            # diagonal stationaries: Sa = (8I)*h2_0 | (8I)*h2_1,
            # Sb = (8I)*h2_2 | 8I   (bias row coefficient)
            stat = []
            for s in range(ns):
                Sa = cp.tile([128, 2, 128], FP8, tag=f"Sa{s}")
                Sb = cp.tile([128, 2, 128], FP8, tag=f"Sb{s}")
                for c in (0, 1):
                    nc.vector.tensor_scalar(
                        Sa[:, c, :], cind[:], h2[:, s, c : c + 1], None,
                        op0=ALU.mult,
                    )
                nc.vector.tensor_scalar(
                    Sb[:, 0, :], cind[:], h2[:, s, 2:3], None, op0=ALU.mult,
                )
                nc.vector.tensor_copy(Sb[:, 1, :], cind[:])
                stat.append((Sa, Sb))

            # ---- persistent psum tiles ------------------------------------
            u_all = pp.tile([NB, O_BLK], F32, tag="uall")
            d_psum = pp.tile([1, m_own], F32, tag="dpsum")

            # ---- main streamed loop ---------------------------------------
            pend = []              # (block, acc) awaiting their reduce matmuls
            for b in range(NB):
                pet = pets[b]
                pev = pet[:, 0:PW].rearrange(
                    "p (s a r c) -> p s a r c", s=ns, a=2, r=2, c=O_BLK)
                wtv = pet[:, PW : PW + DW].rearrange(
                    "p (r m) -> p r m", r=2, m=m_own)
                # ---------- PE path: 2 DoubleRow matmuls per slab -----------
                stks = []
                for s in range(ns):
                    stk = pp.tile([128, O_BLK], F32, tag=f"stk{s}")
                    Sa, Sb = stat[s]
                    nc.tensor.matmul(
                        stk[:], Sa[:], pev[:, s, 0, :, :],
                        start=True, stop=False, perf_mode=PM.DoubleRow,
                    )
                    nc.tensor.matmul(
                        stk[:], Sb[:], pev[:, s, 1, :, :],
                        start=False, stop=True, perf_mode=PM.DoubleRow,
                    )
                    stks.append(stk)

                # ---------- dense matvec: this block's kc chunk -------------
                for mb in range(m_own // 512):
                    lo, hi = mb * 512, (mb + 1) * 512
                    nc.tensor.matmul(
                        d_psum[0:1, lo:hi],
                        xg[:, :, b : b + 1],
                        wtv[:, :, lo:hi],
                        start=(b == 0), stop=(b == NB - 1),
                        perf_mode=PM.DoubleRow,
                    )
                if b == NB - 1:
                    # bias is added on the host
                    dense_sb = outp.tile([1, m_own], F32, tag="dense_sb")
                    nc.vector.tensor_scalar_mul(
                        dense_sb[:], d_psum[:], 1.0 / WT_SCALE)
                    nc.gpsimd.dma_start(d["dense_out"][:], dense_sb[:])

                # ---------- per-slab relus, fully independent ---------------
                # (Scalar takes even slabs, DVE odd slabs; no chaining, so
                # the per-block critical path is one relu past the last MM)
                acc = accp.tile([128, ns, O_BLK], FP8, tag="acc")
                for s in range(ns):
                    if s % 2 == 0:
                        nc.scalar.activation(acc[:, s, :], stks[s][:], AF.Relu)
                    else:
                        nc.vector.tensor_scalar_max(
                            acc[:, s, :], stks[s][:], 0.0)
                pend.append((b, acc))

                # ---------- tail experts, once, early ----------
                if n_tail and b == tail_blk:
                    for e in range(n_tail):
                        tacc = rp.tile([128, OC], BF16, tag=f"tacc{e}")
                        nc.vector.scalar_tensor_tensor(
                            tacc[:], tbf[:, e, 0, :], h2[:, ns + e, 0:1],
                            tbf[:, e, 3, :], op0=ALU.mult, op1=ALU.add,
                        )
                        for i in (1, 2):
                            nc.vector.scalar_tensor_tensor(
                                tacc[:], tbf[:, e, i, :], h2[:, ns + e, i : i + 1],
                                tacc[:], op0=ALU.mult, op1=ALU.add,
                            )
                        if e == 0:
                            nc.scalar.activation(ut[:], tacc[:], AF.Relu)
                        else:
                            rt = rp.tile([128, OC], F32, tag="rt")
                            nc.scalar.activation(rt[:], tacc[:], AF.Relu)
                            nc.vector.tensor_tensor(ut[:], ut[:], rt[:], op=ALU.add)
                    nc.gpsimd.dma_start(d["ut_out"][:], ut[:])

            # ---------- all reduces after the loop + single u store ---------
            # Emitted past every block's matmuls: the scheduler hoists each
            # into Tensor slack once its acc is ready, but a lagging relu
            # can never stall the streaming pipeline.  Two DR matmuls per
            # block contract slab-row pairs (0,1) and (2,3) of acc.
            for pb, pacc in pend:
                for h in range(nrow):
                    nc.tensor.matmul(
                        u_all[:], sel[:, pb, :, :],
                        pacc[:, 2 * h : 2 * h + 2, :],
                        start=(pb == 0 and h == 0),
                        stop=(pb == NB - 1 and h == nrow - 1),
                        perf_mode=PM.DoubleRow,
                    )
            u_sb = outp.tile([NB, O_BLK], F32, tag="u_sb")
            nc.vector.tensor_copy(u_sb[:], u_all[:])
            nc.gpsimd.dma_start(d["u_out"][:], u_sb[:])

    nc.compile()
    return nc, d


def route(inputs):
    """Host-side routing: active experts + active dense columns."""
    x = np.asarray(inputs["x"], dtype=np.float32)
    sat = np.asarray(inputs["saturated"]).astype(bool)
    act = np.nonzero(sat & (x != 0))[0]
    dcols = np.nonzero(~sat)[0]
    per = -(-len(act) // N_CORES)            # ceil
    nslab = per // 128                       # full 128-expert slabs
    if nslab % 2:                            # DR reduce pairs slabs
        nslab -= 1
    n_tail = per - 128 * nslab
    nkc2 = -(-len(dcols) // 256)
    if nkc2 % 2:
        nkc2 += 1                            # dense pair-tiles need even kc
    return act, dcols, per, 0, nslab, n_tail, nkc2


def make_in_maps(inputs, act, dcols, per, nsub, nslab, n_tail, nkc2):
    x = np.asarray(inputs["x"], dtype=np.float32)
    weight = np.asarray(inputs["weight"], dtype=np.float32)
    bias = np.asarray(inputs["bias"], dtype=np.float32)
    W1 = np.asarray(inputs["W1"], dtype=np.float32)
    b1 = np.asarray(inputs["b1"], dtype=np.float32)
    W2 = np.asarray(inputs["W2"], dtype=np.float32)
    b2 = np.asarray(inputs["b2"], dtype=np.float32)
    W3 = np.asarray(inputs["W3"], dtype=np.float32)
    b3 = np.asarray(inputs["b3"], dtype=np.float32)

    ns = nslab
    m_own = SIZE_OUT // N_CORES
    NB = SIZE_OUT // O_BLK
    npair = nkc2 // 2
    n_slab = 128 * ns
    Dp = nkc2 * 256

    W38 = W3.astype(NP_FP8)                  # [N, O, 3]
    b38 = b3.astype(NP_FP8)                  # [N, O]

    xg_full = np.zeros(Dp, dtype=np.float32)
    xg_full[: len(dcols)] = x[dcols]
    # DoubleRow pairs: partition p of chunk kc holds rows kc*256+2p, +1
    xg = np.ascontiguousarray(
        xg_full.reshape(nkc2, 128, 2).transpose(1, 2, 0)
    ).astype(NP_FP8)

    cind = (H2S * np.eye(128, dtype=np.float32)).astype(NP_FP8)

    in_maps = []
    for i in range(N_CORES):
        ids = act[i * per : (i + 1) * per]
        n_live = len(ids)
        if n_live < per:
            ids = np.concatenate([ids, np.zeros(per - n_live, dtype=ids.dtype)])
        gids = ids[:n_slab]
        tids = ids[n_slab:]

        # ---- contiguous per-o-block merged stream tiles ------------------
        G = np.empty((n_slab, SIZE_OUT, 4), dtype=NP_FP8)
        G[:, :, 0:3] = W38[gids]
        G[:, :, 3] = b38[gids]
        live = min(max(n_live, 0), n_slab)
        if live < n_slab:
            G[live:] = 0
        PW = ns * 2 * 2 * O_BLK
        pe = np.empty((NB, 128, PW + 2 * m_own), dtype=NP_FP8)
        pe[:, :, 0:PW] = (
            G.reshape(ns, 128, NB, O_BLK, 4).transpose(2, 1, 0, 4, 3)
            .reshape(NB, 128, PW)
        )

        slm = slice(i * m_own, (i + 1) * m_own)
        wtg = np.zeros((Dp, m_own), dtype=np.float32)
        wtg[: len(dcols)] = weight[slm][:, dcols].T * WT_SCALE
        pe[:, :, PW:] = wtg.astype(NP_FP8).reshape(NB, 128, 2 * m_own)

        # ---- merged small-MLP consts (slab groups + tail groups) ---------
        def grp(a, shp):
            main = a[gids].reshape((ns, 128) + shp).transpose(
                (1, 0) + tuple(range(2, 2 + len(shp))))
            if n_tail:
                tailb = np.broadcast_to(a[tids], (128, n_tail) + shp)
                main = np.concatenate([main, tailb], axis=1)
            return main.reshape(128, -1)

        nt = ns + n_tail
        cpkarr = np.ascontiguousarray(np.concatenate(
            [grp(x, ()), grp(W1, (3, 3)), grp(b1, (3,)),
             grp(W2, (3, 3)), grp(b2, (3,))], axis=1, dtype=np.float32))

        TBF_OFF = 128 + 2 * nkc2
        TBF_W = n_tail * 4 * OC * 2
        CPK_OFF = TBF_OFF + TBF_W
        FPK_W = CPK_OFF + 25 * nt * 4
        raw = np.zeros((128, FPK_W), dtype=np.uint8)
        raw[:, 0:128] = cind.view(np.uint8)
        raw[:, 128:TBF_OFF] = xg.reshape(128, 2 * nkc2).view(np.uint8)
        raw[:, CPK_OFF:FPK_W] = cpkarr.view(np.uint8)

        if n_tail:
            nt_live = max(0, min(n_tail, n_live - n_slab))
            w3tt = np.ascontiguousarray(
                W3[tids]
                .transpose(0, 2, 1)
                .reshape(n_tail, 3, OC, 128)
                .transpose(3, 0, 1, 2)
            ).astype(NP_BF16)
            b3tt = np.ascontiguousarray(
                b3[tids].reshape(n_tail, OC, 128).transpose(2, 0, 1)
            ).astype(NP_BF16)
            if nt_live < n_tail:
                w3tt[:, nt_live:] = 0
                b3tt[:, nt_live:] = 0
            tbf = np.empty((128, n_tail, 4, OC), dtype=NP_BF16)
            tbf[:, :, 0:3, :] = w3tt
            tbf[:, :, 3, :] = b3tt
            raw[:, TBF_OFF:CPK_OFF] = tbf.view(np.uint8).reshape(128, TBF_W)

        m = {"pe": pe, "fpk": raw.view(NP_FP8)}
        in_maps.append(m)
    return in_maps


def combine_outputs(results, names, n_tail, bias=None):
    u = np.zeros(SIZE_OUT, dtype=np.float64)
    dense = []
    for res in results:
        u += res[names["u_out"].name].reshape(-1).astype(np.float64)
        if n_tail:
            ut = res[names["ut_out"].name].astype(np.float64)  # [128, OC]
            u += ut.T.reshape(-1)                              # o = c*128 + p
        dense.append(res[names["dense_out"].name].reshape(-1))
    out = np.concatenate(dense).astype(np.float64) + u
    if bias is not None:
        out = out + np.asarray(bias, dtype=np.float64)
    return out.astype(np.float32)


_CACHE = {}
CONFIG = {}


def _get_program(nsub, nslab, n_tail, nkc2):
    key = (nsub, nslab, n_tail, nkc2, tuple(sorted(CONFIG.items())))
    if key not in _CACHE:
        _CACHE[key] = build_program(nslab, n_tail, nkc2, **CONFIG)
    return _CACHE[key]


def kernel(**inputs):
    act, dcols, per, nsub, nslab, n_tail, nkc2 = route(inputs)
    nc, names = _get_program(nsub, nslab, n_tail, nkc2)
    in_maps = make_in_maps(inputs, act, dcols, per, nsub, nslab, n_tail, nkc2)
    keyed = [{names[k].name: v for k, v in m.items()} for m in in_maps]
    res = run_bass_kernel_spmd(nc, keyed, core_ids=list(range(N_CORES)))
    return combine_outputs(res.results, names, n_tail, inputs["bias"])
